# revision 8
# baseline (speedup 1.0000x reference)
"""Block-sparse linear y = x @ W^T on 8 Trainium2 NeuronCores.

Strategy: the 32x32 block structure (50% block density, random scatter) is not
exploitable on a 128x128 PE array (M=32 tiles run at 25% utilization and the
per-block LDWEIGHTS cost dominates), so we densify W^T on the host (cheap: 8MB
of scatter-adds) and run a dense fp32 GEMM, sharded 4-way over tokens x 2-way
over out_features (8 cores, no collectives needed).

Per core: y_shard[1024, 1024] = xT_shard[2048, 1024]^T @ wT_shard[2048, 1024].
x is transposed on the host so both operands stream into SBUF with K (=in
features) on partitions in natural, fully-contiguous DMA layouts.

Matmul dtype: float32r (single-pass fp32 matmul, 1 cycle/row at N>=512) vs
float32 (2-pass, 4 cycles/row). Switch via MM_DTYPE.
"""

import numpy as np

TOKENS, IN_F, OUT_F = 4096, 2048, 2048
BLOCK = 32
N_CORES = 8
TG, OG = 4, 2  # token groups x out-feature groups
T_SH = TOKENS // TG  # 1024 tokens per core
O_SH = OUT_F // OG  # 1024 out features per core
P = 128
NFREE = 512  # PSUM bank free dim (fp32)
KT = IN_F // P  # 16 k tiles
MT = T_SH // P  # 8 psum row tiles (uses all 8 PSUM banks)
NT = O_SH // NFREE  # 2 out column tiles

MM_DTYPE = "float32r"  # "float32r" (fast) or "float32" (exact 2-pass)
TRACE = False  # set by test.py to capture an NTFF profile

_nc_cache = {}
_last_result = None  # BassKernelResults of the most recent run (for test.py)


def _build_nc():
    import concourse.mybir as mybir
    import concourse.tile as tile
    from concourse import bacc

    key = MM_DTYPE
    if key in _nc_cache:
        return _nc_cache[key]

    dt_mm = getattr(mybir.dt, MM_DTYPE)
    f32 = mybir.dt.float32

    nc = bacc.Bacc(None, target_bir_lowering=False)
    xT = nc.dram_tensor("xT", [IN_F, T_SH], dt_mm, kind="ExternalInput")
    wT = nc.dram_tensor("wT", [IN_F, O_SH], dt_mm, kind="ExternalInput")
    y = nc.dram_tensor("y", [T_SH, O_SH], f32, kind="ExternalOutput")

    # Schedule: quarters (n, kh) over out-halves n and K-halves kh, ordered
    # (0,A)(1,A)(0,B)(1,B); splitting K spreads the 8MB x^T load across the
    # first two quarters (the kernel sits at the DMA roofline). Pass-A psums
    # are evicted to SBUF partials; pass B adds them back on the way out.
    # Each quarter runs as two 4-bank octants (banks 0-3 then 4-7) so psum
    # eviction of one bank set always overlaps matmuls on the other set.
    # Pass B runs k-inner per bank so finished tiles drain immediately.
    # Streams use separate DMA queues (x+out: sync, W: scalar) so one
    # stream's pool-slot wait never blocks the other's FIFO.
    KH = KT // 2  # 8 k-tiles per half
    HB = MT // 2  # 4 banks per octant
    XH = T_SH // 2  # x^T tiles split in halves of 512 tokens
    with tile.TileContext(nc) as tc:
        with (
            tc.tile_pool(name="xp", bufs=1) as xp,
            tc.tile_pool(name="wp", bufs=3) as wp,
            tc.tile_pool(name="pp", bufs=1) as pp,
            tc.tile_pool(name="op", bufs=4) as op,
            tc.tile_pool(name="ps", bufs=1, space="PSUM") as ps,
        ):
            # x^T half-tiles: xh[h][k] covers tokens [h*512, (h+1)*512)
            xh = [[None] * KT, [None] * KT]

            def load_xh(h, k):
                t = xp.tile([P, XH], dt_mm, tag=f"x{h}_{k}", name=f"x{h}_{k}")
                nc.sync.dma_start(
                    t[:], xT[k * P : (k + 1) * P, h * XH : (h + 1) * XH]
                )
                xh[h][k] = t

            def lhsT(m, k):
                return xh[m // HB][k][:, (m % HB) * P : (m % HB + 1) * P]

            partials = {}
            for qi, (n, kh) in enumerate([(0, 0), (1, 0), (0, 1), (1, 1)]):
                # Quarter's 8 W tiles: dedicated slots, live across both
                # octants; bufs=2 lets the next quarter's loads pipeline.
                wts = []
                for ki in range(KH):
                    k = kh * KH + ki
                    wt = wp.tile(
                        [P, NFREE], dt_mm, tag=f"wt{ki}", name=f"wt{ki}"
                    )
                    nc.scalar.dma_start(
                        wt[:], wT[k * P : (k + 1) * P, n * NFREE : (n + 1) * NFREE]
                    )
                    wts.append(wt)
                psums = [
                    ps.tile([P, NFREE], f32, tag=f"ps{m}", name=f"ps{m}")
                    for m in range(MT)
                ]
                if kh == 0:  # pass A: k-outer over all 8 banks, x staged JIT
                    for ki in range(KH):
                        for h in range(2):
                            if qi == 0 and xh[h][ki] is None:
                                load_xh(h, ki)  # A-half of x, just in time
                            if qi == 1 and xh[h][KH + ki] is None:
                                load_xh(h, KH + ki)  # prefetch B half
                        for m in range(MT):
                            nc.tensor.matmul(
                                psums[m][:],
                                lhsT(m, ki),
                                wts[ki][:],
                                start=(ki == 0),
                                stop=(ki == KH - 1),
                            )
                    for m in range(MT):  # evict partial sums to SBUF
                        pt = pp.tile(
                            [P, NFREE], f32, tag=f"pt{n}_{m}", name=f"pt{n}_{m}"
                        )
                        nc.vector.tensor_copy(pt[:], psums[m][:])
                        partials[(n, m)] = pt
                else:  # pass B: k-inner per bank so finished banks drain early
                    for m in range(MT):
                        for ki in range(KH):
                            nc.tensor.matmul(
                                psums[m][:],
                                lhsT(m, KH + ki),
                                wts[ki][:],
                                start=(ki == 0),
                                stop=(ki == KH - 1),
                            )
                        ot = op.tile([P, NFREE], f32, tag="ot")
                        nc.vector.tensor_add(
                            out=ot[:], in0=psums[m][:], in1=partials[(n, m)][:]
                        )
                        nc.sync.dma_start(
                            y[m * P : (m + 1) * P, n * NFREE : (n + 1) * NFREE],
                            ot[:],
                        )

    nc.compile()
    _nc_cache[key] = nc
    return nc


def _densify_wT(weight_blocks, block_rows, block_cols):
    """Scatter-add the 32x32 blocks into dense W^T [in_features, out_features]."""
    nc_blk = IN_F // BLOCK
    nr_blk = OUT_F // BLOCK
    wcr = np.zeros((nc_blk, nr_blk, BLOCK, BLOCK), np.float32)
    # block b occupies W[32r:32r+32, 32c:32c+32]; W^T gets the transposed block
    np.add.at(
        wcr,
        (block_cols.astype(np.int64), block_rows.astype(np.int64)),
        np.swapaxes(weight_blocks.astype(np.float32, copy=False), 1, 2),
    )
    return np.ascontiguousarray(wcr.transpose(0, 2, 1, 3).reshape(IN_F, OUT_F))


def kernel(x, weight_blocks, block_rows, block_cols):
    global _last_result
    from concourse.bass_utils import run_bass_kernel_spmd

    x = np.asarray(x, dtype=np.float32)
    wT = _densify_wT(
        np.asarray(weight_blocks), np.asarray(block_rows), np.asarray(block_cols)
    )
    xT = np.ascontiguousarray(x.T)

    in_maps = []
    for c in range(N_CORES):
        tg, og = divmod(c, OG)
        in_maps.append(
            {
                "xT": np.ascontiguousarray(xT[:, tg * T_SH : (tg + 1) * T_SH]),
                "wT": np.ascontiguousarray(wT[:, og * O_SH : (og + 1) * O_SH]),
            }
        )

    nc = _build_nc()
    res = run_bass_kernel_spmd(
        nc, in_maps, core_ids=list(range(N_CORES)), trace=TRACE
    )
    _last_result = res

    y = np.empty((TOKENS, OUT_F), np.float32)
    for c in range(N_CORES):
        tg, og = divmod(c, OG)
        y[tg * T_SH : (tg + 1) * T_SH, og * O_SH : (og + 1) * O_SH] = res.results[c][
            "y"
        ]
    return y


# revision 9
# speedup vs baseline: 1.0648x; 1.0648x over previous
"""Block-sparse linear y = x @ W^T on 8 Trainium2 NeuronCores.

Strategy: the 32x32 block structure (50% block density, random scatter) is not
exploitable on a 128x128 PE array (M=32 tiles run at 25% utilization and the
per-block LDWEIGHTS cost dominates), so we densify W^T on the host (cheap: 8MB
of scatter-adds) and run a dense fp32 GEMM, sharded 4-way over tokens x 2-way
over out_features (8 cores, no collectives needed).

Per core: y_shard[1024, 1024] = xT_shard[2048, 1024]^T @ wT_shard[2048, 1024].
x is transposed on the host so both operands stream into SBUF with K (=in
features) on partitions in natural, fully-contiguous DMA layouts.

Matmul dtype: float32r (single-pass fp32 matmul, 1 cycle/row at N>=512) vs
float32 (2-pass, 4 cycles/row). Switch via MM_DTYPE.
"""

import numpy as np

TOKENS, IN_F, OUT_F = 4096, 2048, 2048
BLOCK = 32
N_CORES = 8
TG, OG = 4, 2  # token groups x out-feature groups
T_SH = TOKENS // TG  # 1024 tokens per core
O_SH = OUT_F // OG  # 1024 out features per core
P = 128
NFREE = 512  # PSUM bank free dim (fp32)
KT = IN_F // P  # 16 k tiles
MT = T_SH // P  # 8 psum row tiles (uses all 8 PSUM banks)
NT = O_SH // NFREE  # 2 out column tiles

MM_DTYPE = "float32r"  # "float32r" (fast) or "float32" (exact 2-pass)
TRACE = False  # set by test.py to capture an NTFF profile

_nc_cache = {}
_last_result = None  # BassKernelResults of the most recent run (for test.py)


def _build_nc():
    import concourse.mybir as mybir
    import concourse.tile as tile
    from concourse import bacc

    key = MM_DTYPE
    if key in _nc_cache:
        return _nc_cache[key]

    dt_mm = getattr(mybir.dt, MM_DTYPE)
    f32 = mybir.dt.float32

    nc = bacc.Bacc(None, target_bir_lowering=False)
    xT = nc.dram_tensor("xT", [IN_F, T_SH], dt_mm, kind="ExternalInput")
    wT = nc.dram_tensor("wT", [IN_F, O_SH], dt_mm, kind="ExternalInput")
    y = nc.dram_tensor("y", [T_SH, O_SH], f32, kind="ExternalOutput")

    # Schedule: quarters (n, kh) over out-halves n and K-halves kh, ordered
    # (0,A)(1,A)(0,B)(1,B); splitting K spreads the 8MB x^T load across the
    # first two quarters (the kernel sits at the DMA roofline). Pass-A psums
    # are evicted to SBUF partials; pass B adds them back on the way out.
    # Each quarter runs as two 4-bank octants (banks 0-3 then 4-7) so psum
    # eviction of one bank set always overlaps matmuls on the other set.
    # Pass B runs k-inner per bank so finished tiles drain immediately.
    # Streams use separate DMA queues (x+out: sync, W: scalar) so one
    # stream's pool-slot wait never blocks the other's FIFO.
    KH = KT // 2  # 8 k-tiles per half
    HB = MT // 2  # 4 token-half banks
    XH = T_SH // 2  # x^T tiles split in halves of 512 tokens
    # Views with the k-tile dim explicit, for multi-k-tile strided DMAs
    wT3 = wT.rearrange("(ko p) f -> p ko f", p=P)
    xT3 = xT.rearrange("(ko p) f -> p ko f", p=P)
    with tile.TileContext(nc) as tc:
        with (
            tc.tile_pool(name="xp", bufs=1) as xp,
            tc.tile_pool(name="wp", bufs=1) as wp,
            tc.tile_pool(name="pp", bufs=1) as pp,
            tc.tile_pool(name="op", bufs=4) as op,
            tc.tile_pool(name="ps", bufs=1, space="PSUM") as ps,
        ):
            # First quarter: per-k JIT tiles (x halves + W), so the first
            # matmul fires as early as possible. Later quarters: one big
            # strided DMA per stream (8 k-tiles at once) — far fewer DMAs,
            # no semaphore-ring/slot-wait serialization on the issue path.
            xhA = [[None] * KH, [None] * KH]  # A-half x: [h][ki] -> [P, XH]
            xbs = [None, None]  # B-half x supertiles: [h] -> [P, KH, XH]
            wq = [None] * 4  # per-quarter W: q0 -> list of [P,NFREE]; else [P,KH,NFREE]

            wts0 = []
            for ki in range(KH):
                wt = wp.tile([P, NFREE], dt_mm, tag=f"wt{ki}", name=f"wt{ki}")
                nc.scalar.dma_start(wt[:], wT[ki * P : (ki + 1) * P, 0:NFREE])
                wts0.append(wt)
            wq[0] = wts0

            def rhs(qi, ki):
                return wq[qi][ki][:] if qi == 0 else wq[qi][:, ki, :]

            def lhsT(m, kh, ki):
                h, ms = m // HB, m % HB
                if kh == 0:
                    return xhA[h][ki][:, ms * P : (ms + 1) * P]
                return xbs[h][:, ki, ms * P : (ms + 1) * P]

            partials = {}
            for qi, (n, kh) in enumerate([(0, 0), (1, 0), (0, 1), (1, 1)]):
                if qi > 0:  # load this quarter's 8 W k-tiles in one DMA
                    w = wp.tile(
                        [P, KH, NFREE], dt_mm, tag=f"wq{qi % 2}", name=f"wq{qi}"
                    )
                    nc.scalar.dma_start(
                        w[:],
                        wT3[:, kh * KH : (kh + 1) * KH, n * NFREE : (n + 1) * NFREE],
                    )
                    wq[qi] = w
                if qi == 1:  # prefetch the B half of x, one DMA per token-half
                    for h in range(2):
                        xb = xp.tile(
                            [P, KH, XH], dt_mm, tag=f"xb{h}", name=f"xb{h}"
                        )
                        nc.sync.dma_start(
                            xb[:], xT3[:, KH:KT, h * XH : (h + 1) * XH]
                        )
                        xbs[h] = xb
                psums = [
                    ps.tile([P, NFREE], f32, tag=f"ps{m}", name=f"ps{m}")
                    for m in range(MT)
                ]
                if kh == 0:  # pass A: k-outer over all 8 banks, x staged JIT
                    for ki in range(KH):
                        if qi == 0:
                            for h in range(2):
                                t = xp.tile(
                                    [P, XH], dt_mm, tag=f"x{h}_{ki}", name=f"x{h}_{ki}"
                                )
                                nc.sync.dma_start(
                                    t[:],
                                    xT[ki * P : (ki + 1) * P, h * XH : (h + 1) * XH],
                                )
                                xhA[h][ki] = t
                        for m in range(MT):
                            nc.tensor.matmul(
                                psums[m][:],
                                lhsT(m, 0, ki),
                                rhs(qi, ki),
                                start=(ki == 0),
                                stop=(ki == KH - 1),
                            )
                    for m in range(MT):  # evict partial sums to SBUF
                        pt = pp.tile(
                            [P, NFREE], f32, tag=f"pt{n}_{m}", name=f"pt{n}_{m}"
                        )
                        nc.vector.tensor_copy(pt[:], psums[m][:])
                        partials[(n, m)] = pt
                else:  # pass B: k-inner per bank so finished banks drain early
                    for m in range(MT):
                        for ki in range(KH):
                            nc.tensor.matmul(
                                psums[m][:],
                                lhsT(m, 1, ki),
                                rhs(qi, ki),
                                start=(ki == 0),
                                stop=(ki == KH - 1),
                            )
                        ot = op.tile([P, NFREE], f32, tag="ot")
                        nc.vector.tensor_add(
                            out=ot[:], in0=psums[m][:], in1=partials[(n, m)][:]
                        )
                        nc.sync.dma_start(
                            y[m * P : (m + 1) * P, n * NFREE : (n + 1) * NFREE],
                            ot[:],
                        )

    nc.compile()
    _nc_cache[key] = nc
    return nc


def _densify_wT(weight_blocks, block_rows, block_cols):
    """Scatter-add the 32x32 blocks into dense W^T [in_features, out_features]."""
    nc_blk = IN_F // BLOCK
    nr_blk = OUT_F // BLOCK
    wcr = np.zeros((nc_blk, nr_blk, BLOCK, BLOCK), np.float32)
    # block b occupies W[32r:32r+32, 32c:32c+32]; W^T gets the transposed block
    np.add.at(
        wcr,
        (block_cols.astype(np.int64), block_rows.astype(np.int64)),
        np.swapaxes(weight_blocks.astype(np.float32, copy=False), 1, 2),
    )
    return np.ascontiguousarray(wcr.transpose(0, 2, 1, 3).reshape(IN_F, OUT_F))


def kernel(x, weight_blocks, block_rows, block_cols):
    global _last_result
    from concourse.bass_utils import run_bass_kernel_spmd

    x = np.asarray(x, dtype=np.float32)
    wT = _densify_wT(
        np.asarray(weight_blocks), np.asarray(block_rows), np.asarray(block_cols)
    )
    xT = np.ascontiguousarray(x.T)

    in_maps = []
    for c in range(N_CORES):
        tg, og = divmod(c, OG)
        in_maps.append(
            {
                "xT": np.ascontiguousarray(xT[:, tg * T_SH : (tg + 1) * T_SH]),
                "wT": np.ascontiguousarray(wT[:, og * O_SH : (og + 1) * O_SH]),
            }
        )

    nc = _build_nc()
    res = run_bass_kernel_spmd(
        nc, in_maps, core_ids=list(range(N_CORES)), trace=TRACE
    )
    _last_result = res

    y = np.empty((TOKENS, OUT_F), np.float32)
    for c in range(N_CORES):
        tg, og = divmod(c, OG)
        y[tg * T_SH : (tg + 1) * T_SH, og * O_SH : (og + 1) * O_SH] = res.results[c][
            "y"
        ]
    return y


# revision 12
# speedup vs baseline: 1.1104x; 1.0428x over previous
"""Block-sparse linear y = x @ W^T on 8 Trainium2 NeuronCores.

Strategy: the 32x32 block structure (50% block density, random scatter) is not
exploitable on a 128x128 PE array (M=32 tiles run at 25% utilization and the
per-block LDWEIGHTS cost dominates), so we densify W^T on the host (cheap: 8MB
of scatter-adds) and run a dense GEMM, sharded 4-way over tokens x 2-way over
out_features (8 cores, no collectives). That sharding minimizes per-core HBM
traffic (20MB: x^T 8 + W^T 8 + y 4) — the kernel sits right at the ridge of
the DMA (~20MB / ~360GB/s) and PE (256 matmuls x ~227ns) rooflines.

Matmuls run in float32r (single-pass fp32 mode, full PE rate at N=512,
~1.2e-4 scale-relative error vs the fp32 reference). MM_DTYPE="float32"
switches to exact 2-pass fp32 at 4x the PE cost.

Schedule per core: out-halves n x K-halves kh as quarters (0A)(1A)(0B)(1B).
Splitting K spreads the x^T load across the first two quarters; pass-A psums
are evicted to SBUF partials, pass B adds them back while draining banks
k-inner so outputs stream out early. All input tensors are pre-blocked on the
host into the exact SBUF layouts, so every load is a single linear DMA with
16KB-contiguous runs; the data stream (x + W, sync queue) is emitted in
consumption-priority order and outputs go on the scalar queue so neither
stream's waits can block the other.
"""

import numpy as np

TOKENS, IN_F, OUT_F = 4096, 2048, 2048
BLOCK = 32
N_CORES = 8
TG, OG = 4, 2  # token groups x out-feature groups
T_SH = TOKENS // TG  # 1024 tokens per core
O_SH = OUT_F // OG  # 1024 out features per core
P = 128
NFREE = 512  # PSUM bank free dim (fp32)
KT = IN_F // P  # 16 k tiles
MT = T_SH // P  # 8 psum banks
NT = O_SH // NFREE  # 2 out column tiles
KH = KT // 2  # 8 k-tiles per K-half
XH = T_SH // 2  # token half
QUARTERS = [(0, 0), (1, 0), (0, 1), (1, 1)]  # (out-half n, K-half kh)

MM_DTYPE = "float32r"  # "float32r" (fast) or "float32" (exact 2-pass)
TRACE = False  # set by test.py to capture an NTFF profile

_nc_cache = {}
_last_result = None  # BassKernelResults of the most recent run (for test.py)


def _build_nc():
    import concourse.mybir as mybir
    import concourse.tile as tile
    from concourse import bacc

    key = MM_DTYPE
    if key in _nc_cache:
        return _nc_cache[key]

    dt_mm = getattr(mybir.dt, MM_DTYPE)
    f32 = mybir.dt.float32

    nc = bacc.Bacc(None, target_bir_lowering=False)
    # Host-pre-blocked inputs (exact SBUF layouts; all DMAs are linear):
    # xa: A-half x^T k-tiles, [KH][P][T_SH]
    # xb: B-half x^T supertiles by token-half, [2][P][KH][XH]
    # wq: per-quarter W^T supertiles, [4][P][KH][NFREE]
    xa = nc.dram_tensor("xa", [KH, P, T_SH], dt_mm, kind="ExternalInput")
    xb = nc.dram_tensor("xb", [2, P, KH, XH], dt_mm, kind="ExternalInput")
    wq4 = nc.dram_tensor("wq", [4, P, KH, NFREE], dt_mm, kind="ExternalInput")
    y = nc.dram_tensor("y", [T_SH, O_SH], f32, kind="ExternalOutput")

    with tile.TileContext(nc) as tc:
        with (
            tc.tile_pool(name="xp", bufs=1) as xp,
            tc.tile_pool(name="wp", bufs=1) as wp,
            tc.tile_pool(name="pp", bufs=1) as pp,
            tc.tile_pool(name="op", bufs=4) as op,
            tc.tile_pool(name="ps", bufs=1, space="PSUM") as ps,
        ):
            xa_t = [None] * KH  # [P, T_SH] tiles (ki=0 split in two halves)
            xa0 = [None, None]
            xbs = [None, None]  # [P, KH, XH] supertiles
            wq = [[], None, None, None]  # q0: list of [P, NFREE]; q1+: [P, KH, NFREE]

            def lhsT(m, kh, ki):
                if kh == 1:
                    return xbs[m // 4][:, ki, (m % 4) * P : (m % 4 + 1) * P]
                if ki == 0:
                    return xa0[m // 4][:, (m % 4) * P : (m % 4 + 1) * P]
                return xa_t[ki][:, m * P : (m + 1) * P]

            def rhs(qi, ki):
                return wq[qi][ki][:] if qi == 0 else wq[qi][:, ki, :]

            partials = {}
            for qi, (n, kh) in enumerate(QUARTERS):
                if qi > 0:  # this quarter's W: one linear 2MB DMA
                    w = wp.tile(
                        [P, KH, NFREE], dt_mm, tag=f"wq{qi % 2}", name=f"wq{qi}"
                    )
                    nc.sync.dma_start(w[:], wq4[qi])
                    wq[qi] = w
                if qi == 2:  # B-half x: two linear 2MB DMAs (after wq1+wq2)
                    for h in range(2):
                        t = xp.tile([P, KH, XH], dt_mm, tag=f"xb{h}", name=f"xb{h}")
                        nc.sync.dma_start(t[:], xb[h])
                        xbs[h] = t
                psums = [
                    ps.tile([P, NFREE], f32, tag=f"ps{m}", name=f"ps{m}")
                    for m in range(MT)
                ]
                if kh == 0:  # pass A: k-outer over all 8 banks, x staged JIT
                    for ki in range(KH):
                        if qi == 0:
                            if ki == 0:  # halve the first tile: earliest MM
                                for h in range(2):
                                    t = xp.tile(
                                        [P, XH], dt_mm, tag=f"xa0{h}", name=f"xa0{h}"
                                    )
                                    nc.sync.dma_start(
                                        t[:], xa[0, :, h * XH : (h + 1) * XH]
                                    )
                                    xa0[h] = t
                            else:
                                t = xp.tile(
                                    [P, T_SH], dt_mm, tag=f"xa{ki}", name=f"xa{ki}"
                                )
                                nc.sync.dma_start(t[:], xa[ki])
                                xa_t[ki] = t
                            wt = wp.tile(
                                [P, NFREE], dt_mm, tag=f"wt{ki}", name=f"wt{ki}"
                            )
                            nc.sync.dma_start(wt[:], wq4[0, :, ki, :])
                            wq[0].append(wt)
                        for m in range(MT):
                            nc.tensor.matmul(
                                psums[m][:],
                                lhsT(m, 0, ki),
                                rhs(qi, ki),
                                start=(ki == 0),
                                stop=(ki == KH - 1),
                            )
                    for m in range(MT):  # evict partial sums to SBUF
                        pt = pp.tile(
                            [P, NFREE], f32, tag=f"pt{n}_{m}", name=f"pt{n}_{m}"
                        )
                        nc.vector.tensor_copy(pt[:], psums[m][:])
                        partials[(n, m)] = pt
                else:  # pass B: k-inner per bank so finished banks drain early
                    for m in range(MT):
                        for ki in range(KH):
                            nc.tensor.matmul(
                                psums[m][:],
                                lhsT(m, 1, ki),
                                rhs(qi, ki),
                                start=(ki == 0),
                                stop=(ki == KH - 1),
                            )
                        ot = op.tile([P, NFREE], f32, tag="ot")
                        nc.vector.tensor_add(
                            out=ot[:], in0=psums[m][:], in1=partials[(n, m)][:]
                        )
                        nc.scalar.dma_start(
                            y[m * P : (m + 1) * P, n * NFREE : (n + 1) * NFREE],
                            ot[:],
                        )

    nc.compile()
    _nc_cache[key] = nc
    return nc


def _densify_wT(weight_blocks, block_rows, block_cols):
    """Scatter-add the 32x32 blocks into dense W^T [in_features, out_features]."""
    nc_blk = IN_F // BLOCK
    nr_blk = OUT_F // BLOCK
    wcr = np.zeros((nc_blk, nr_blk, BLOCK, BLOCK), np.float32)
    # block b occupies W[32r:32r+32, 32c:32c+32]; W^T gets the transposed block
    np.add.at(
        wcr,
        (block_cols.astype(np.int64), block_rows.astype(np.int64)),
        np.swapaxes(weight_blocks.astype(np.float32, copy=False), 1, 2),
    )
    return np.ascontiguousarray(wcr.transpose(0, 2, 1, 3).reshape(IN_F, OUT_F))


def _pack_core_inputs(xT_sh, wT_sh):
    """Block one core's x^T and W^T shards into the kernel's DMA layouts."""
    Xsh = xT_sh.reshape(KT, P, T_SH)
    xa = np.ascontiguousarray(Xsh[:KH])  # [KH, P, T_SH]
    xb = np.ascontiguousarray(  # [2, P, KH, XH]
        np.stack(
            [
                Xsh[KH:, :, :XH].transpose(1, 0, 2),
                Xsh[KH:, :, XH:].transpose(1, 0, 2),
            ]
        )
    )
    Wsh = wT_sh.reshape(2, KH, P, O_SH)  # [kh, ki, p, f]
    wq = np.ascontiguousarray(  # [4, P, KH, NFREE], quarter order
        np.stack(
            [
                Wsh[kh, :, :, n * NFREE : (n + 1) * NFREE].transpose(1, 0, 2)
                for (n, kh) in QUARTERS
            ]
        )
    )
    return {"xa": xa, "xb": xb, "wq": wq}


def kernel(x, weight_blocks, block_rows, block_cols):
    global _last_result
    from concourse.bass_utils import run_bass_kernel_spmd

    x = np.asarray(x, dtype=np.float32)
    wT = _densify_wT(
        np.asarray(weight_blocks), np.asarray(block_rows), np.asarray(block_cols)
    )
    xT = np.ascontiguousarray(x.T)

    in_maps = []
    for c in range(N_CORES):
        tg, og = divmod(c, OG)
        in_maps.append(
            _pack_core_inputs(
                xT[:, tg * T_SH : (tg + 1) * T_SH],
                wT[:, og * O_SH : (og + 1) * O_SH],
            )
        )

    nc = _build_nc()
    res = run_bass_kernel_spmd(nc, in_maps, core_ids=list(range(N_CORES)), trace=TRACE)
    _last_result = res

    y = np.empty((TOKENS, OUT_F), np.float32)
    for c in range(N_CORES):
        tg, og = divmod(c, OG)
        y[tg * T_SH : (tg + 1) * T_SH, og * O_SH : (og + 1) * O_SH] = res.results[c][
            "y"
        ]
    return y


# revision 16
# speedup vs baseline: 1.1632x; 1.0475x over previous
"""Block-sparse linear y = x @ W^T on 8 Trainium2 NeuronCores.

Strategy: the 32x32 block structure (50% block density, random scatter) is not
exploitable on a 128x128 PE array (M=32 tiles run at 25% utilization and the
per-block LDWEIGHTS cost dominates), so we densify W^T on the host (cheap: 8MB
of scatter-adds) and run a dense GEMM, sharded 4-way over tokens x 2-way over
out_features (8 cores, no collectives). That sharding minimizes per-core HBM
traffic (20MB: x^T 8 + W^T 8 + y 4) — the kernel sits right at the ridge of
the DMA (~20MB / ~360GB/s) and PE (256 matmuls x ~227ns) rooflines.

Matmuls run in float32r (single-pass fp32 mode, full PE rate at N=512,
~1.2e-4 scale-relative error vs the fp32 reference). MM_DTYPE="float32"
switches to exact 2-pass fp32 at 4x the PE cost.

Schedule per core: out-halves n x K-halves kh as quarters (0A)(1A)(0B)(1B).
Splitting K spreads the x^T load across the first two quarters; pass-A psums
are evicted to SBUF partials, pass B adds them back while draining banks
k-inner so outputs stream out early. All input tensors are pre-blocked on the
host into the exact SBUF layouts, so every load is a single linear DMA with
16KB-contiguous runs; the data stream (x + W, sync queue) is emitted in
consumption-priority order and outputs go on the scalar queue so neither
stream's waits can block the other.
"""

import numpy as np

TOKENS, IN_F, OUT_F = 4096, 2048, 2048
BLOCK = 32
N_CORES = 8
TG, OG = 4, 2  # token groups x out-feature groups
T_SH = TOKENS // TG  # 1024 tokens per core
O_SH = OUT_F // OG  # 1024 out features per core
P = 128
NFREE = 512  # PSUM bank free dim (fp32)
KT = IN_F // P  # 16 k tiles
MT = T_SH // P  # 8 psum banks
NT = O_SH // NFREE  # 2 out column tiles
KH = KT // 2  # 8 k-tiles per K-half
XH = T_SH // 2  # token half
QUARTERS = [(0, 0), (1, 0), (0, 1), (1, 1)]  # (out-half n, K-half kh)

MM_DTYPE = "float32r"  # "float32r" (fast) or "float32" (exact 2-pass)
TRACE = False  # set by test.py to capture an NTFF profile

_nc_cache = {}
_last_result = None  # BassKernelResults of the most recent run (for test.py)


def _build_nc():
    import concourse.mybir as mybir
    import concourse.tile as tile
    from concourse import bacc

    key = MM_DTYPE
    if key in _nc_cache:
        return _nc_cache[key]

    dt_mm = getattr(mybir.dt, MM_DTYPE)
    f32 = mybir.dt.float32

    nc = bacc.Bacc(None, target_bir_lowering=False)
    # Host-pre-blocked inputs (exact SBUF layouts; all DMAs are linear):
    # xa: A-half x^T k-tiles, [KH][P][T_SH]
    # xb: B-half x^T supertiles by token-half, [2][P][KH][XH]
    # wq: per-quarter W^T supertiles, [4][P][KH][NFREE]
    xa = nc.dram_tensor("xa", [KH, P, T_SH], dt_mm, kind="ExternalInput")
    xb = nc.dram_tensor("xb", [2, P, KH, XH], dt_mm, kind="ExternalInput")
    wq4 = nc.dram_tensor("wq", [4, P, KH, NFREE], dt_mm, kind="ExternalInput")
    y = nc.dram_tensor("y", [T_SH, O_SH], f32, kind="ExternalOutput")

    with tile.TileContext(nc) as tc:
        with (
            tc.tile_pool(name="xp", bufs=1) as xp,
            tc.tile_pool(name="wp", bufs=1) as wp,
            tc.tile_pool(name="pp", bufs=1) as pp,
            tc.tile_pool(name="op", bufs=4) as op,
            tc.tile_pool(name="ps", bufs=1, space="PSUM") as ps,
        ):
            xa_t = [None] * KH  # [P, T_SH] tiles (ki=0 split in two halves)
            xa0 = [None, None]
            xbs = [None, None]  # [P, KH, XH] supertiles
            wq = [[], [], None, None]  # pass A: lists of [P, NFREE]; pass B: [P, KH, NFREE]

            def lhsT(m, kh, ki):
                if kh == 1:
                    return xbs[m // 4][:, ki, (m % 4) * P : (m % 4 + 1) * P]
                if ki == 0:
                    return xa0[m // 4][:, (m % 4) * P : (m % 4 + 1) * P]
                return xa_t[ki][:, m * P : (m + 1) * P]

            def rhs(qi, ki):
                return wq[qi][ki][:] if qi <= 1 else wq[qi][:, ki, :]

            partials = {}
            for qi, (n, kh) in enumerate(QUARTERS):
                if qi >= 2:  # pass-B quarters: W as one linear 2MB DMA
                    w = wp.tile(
                        [P, KH, NFREE], dt_mm, tag=f"wq{qi % 2}", name=f"wq{qi}"
                    )
                    nc.sync.dma_start(w[:], wq4[qi])
                    wq[qi] = w
                if qi == 2:  # B-half x: two linear 2MB DMAs (after wq2)
                    for h in range(2):
                        t = xp.tile([P, KH, XH], dt_mm, tag=f"xb{h}", name=f"xb{h}")
                        nc.sync.dma_start(t[:], xb[h])
                        xbs[h] = t
                psums = [
                    ps.tile([P, NFREE], f32, tag=f"ps{m}", name=f"ps{m}")
                    for m in range(MT)
                ]
                if kh == 0:  # pass A: k-outer over all 8 banks, x staged JIT
                    for ki in range(KH):
                        if qi == 0:
                            if ki == 0:  # halve the first tile: earliest MM
                                for h in range(2):
                                    t = xp.tile(
                                        [P, XH], dt_mm, tag=f"xa0{h}", name=f"xa0{h}"
                                    )
                                    nc.sync.dma_start(
                                        t[:], xa[0, :, h * XH : (h + 1) * XH]
                                    )
                                    xa0[h] = t
                            else:
                                t = xp.tile(
                                    [P, T_SH], dt_mm, tag=f"xa{ki}", name=f"xa{ki}"
                                )
                                nc.sync.dma_start(t[:], xa[ki])
                                xa_t[ki] = t
                        # pass-A W: per-k JIT tiles, consumed k-serially
                        wt = wp.tile(
                            [P, NFREE], dt_mm, tag=f"wt{qi}_{ki}", name=f"wt{qi}_{ki}"
                        )
                        nc.sync.dma_start(wt[:], wq4[qi, :, ki, :])
                        wq[qi].append(wt)
                        for m in range(MT):
                            nc.tensor.matmul(
                                psums[m][:],
                                lhsT(m, 0, ki),
                                rhs(qi, ki),
                                start=(ki == 0),
                                stop=(ki == KH - 1),
                            )
                    for m in range(MT):  # evict partial sums to SBUF
                        pt = pp.tile(
                            [P, NFREE], f32, tag=f"pt{n}_{m}", name=f"pt{n}_{m}"
                        )
                        nc.vector.tensor_copy(pt[:], psums[m][:])
                        partials[(n, m)] = pt
                else:  # pass B: k-inner per bank so finished banks drain early
                    for m in range(MT):
                        for ki in range(KH):
                            nc.tensor.matmul(
                                psums[m][:],
                                lhsT(m, 1, ki),
                                rhs(qi, ki),
                                start=(ki == 0),
                                stop=(ki == KH - 1),
                            )
                        ot = op.tile([P, NFREE], f32, tag="ot")
                        nc.vector.tensor_add(
                            out=ot[:], in0=psums[m][:], in1=partials[(n, m)][:]
                        )
                        nc.scalar.dma_start(
                            y[m * P : (m + 1) * P, n * NFREE : (n + 1) * NFREE],
                            ot[:],
                        )

    nc.compile()
    _nc_cache[key] = nc
    return nc


def _densify_wT(weight_blocks, block_rows, block_cols):
    """Scatter-add the 32x32 blocks into dense W^T [in_features, out_features]."""
    nc_blk = IN_F // BLOCK
    nr_blk = OUT_F // BLOCK
    wcr = np.zeros((nc_blk, nr_blk, BLOCK, BLOCK), np.float32)
    # block b occupies W[32r:32r+32, 32c:32c+32]; W^T gets the transposed block
    np.add.at(
        wcr,
        (block_cols.astype(np.int64), block_rows.astype(np.int64)),
        np.swapaxes(weight_blocks.astype(np.float32, copy=False), 1, 2),
    )
    return np.ascontiguousarray(wcr.transpose(0, 2, 1, 3).reshape(IN_F, OUT_F))


def _pack_core_inputs(xT_sh, wT_sh):
    """Block one core's x^T and W^T shards into the kernel's DMA layouts."""
    Xsh = xT_sh.reshape(KT, P, T_SH)
    xa = np.ascontiguousarray(Xsh[:KH])  # [KH, P, T_SH]
    xb = np.ascontiguousarray(  # [2, P, KH, XH]
        np.stack(
            [
                Xsh[KH:, :, :XH].transpose(1, 0, 2),
                Xsh[KH:, :, XH:].transpose(1, 0, 2),
            ]
        )
    )
    Wsh = wT_sh.reshape(2, KH, P, O_SH)  # [kh, ki, p, f]
    wq = np.ascontiguousarray(  # [4, P, KH, NFREE], quarter order
        np.stack(
            [
                Wsh[kh, :, :, n * NFREE : (n + 1) * NFREE].transpose(1, 0, 2)
                for (n, kh) in QUARTERS
            ]
        )
    )
    return {"xa": xa, "xb": xb, "wq": wq}


def kernel(x, weight_blocks, block_rows, block_cols):
    global _last_result
    from concourse.bass_utils import run_bass_kernel_spmd

    x = np.asarray(x, dtype=np.float32)
    wT = _densify_wT(
        np.asarray(weight_blocks), np.asarray(block_rows), np.asarray(block_cols)
    )
    xT = np.ascontiguousarray(x.T)

    in_maps = []
    for c in range(N_CORES):
        tg, og = divmod(c, OG)
        in_maps.append(
            _pack_core_inputs(
                xT[:, tg * T_SH : (tg + 1) * T_SH],
                wT[:, og * O_SH : (og + 1) * O_SH],
            )
        )

    nc = _build_nc()
    res = run_bass_kernel_spmd(nc, in_maps, core_ids=list(range(N_CORES)), trace=TRACE)
    _last_result = res

    y = np.empty((TOKENS, OUT_F), np.float32)
    for c in range(N_CORES):
        tg, og = divmod(c, OG)
        y[tg * T_SH : (tg + 1) * T_SH, og * O_SH : (og + 1) * O_SH] = res.results[c][
            "y"
        ]
    return y
